# revision 16
# baseline (speedup 1.0000x reference)
"""Trainium2 Bass kernel for nn_MiniLSTMParallelCell.

Data-parallel over batch B=8 across 8 NeuronCores (1 sample/core).

Math (per sample, all on-device in [d_out=partition, t=free] layout):
  zi/zf/zh = x @ W^T + b   (PE, float32r, K=1024 contraction)
  p=e^{-zi}, q=e^{-zf}, u=e^{-zh}           (ACT Exp, bias=-b folded in)
  A = ln(1+p) = softplus(-zi)
  C = ln(2+p+q)
  log_f = A - C; c[d,t] = cumsum_d(log_f)    (PE fp32 triangular + rank-1 carry)
  i_gate = (1+q)/(2+p+q)                     (DVE reciprocal_approx_fast)
  g = relu(zh+bh) + 1/(1+max(u,1))           (= minLSTM log_g in linear space)
  iv = i_gate * g
  h[t] = exp(c[t]+m)*(S0 + sum_{s<=t} exp(-c[s]-m)*iv[s])   m = per-d shift
  time-cumsum via DVE tensor_tensor_scan along the free dim.

Host does: x/W transposes, g(h_0), output detranspose (not part of HW time).
"""
import os
import sys
import numpy as np

sys.path.insert(0, "/opt/trn_rl_repo")

from contextlib import ExitStack

import concourse.bass as bass
import concourse.bacc as bacc
import concourse.tile as tile
import concourse.mybir as mybir
from concourse.bass_utils import run_bass_kernel_spmd

F32 = mybir.dt.float32
F32R = mybir.dt.float32r
AF = mybir.ActivationFunctionType
ALU = mybir.AluOpType
AX = mybir.AxisListType

B, T, D = 8, 4096, 1024
CH = 512                  # time chunk (PSUM bank width)
NCH = T // CH             # 8 chunks
OT = D // 128             # 8 output-feature tiles
KT = D // 128             # 8 contraction tiles

_compiled_nc = None


def build_nc():
    nc = bacc.Bacc("TRN2", debug=False, target_bir_lowering=False)

    d_xT = nc.dram_tensor("xT", [D, T], F32R, kind="ExternalInput").ap()
    d_wiT = nc.dram_tensor("wiT", [D, D], F32R, kind="ExternalInput").ap()
    d_wfT = nc.dram_tensor("wfT", [D, D], F32R, kind="ExternalInput").ap()
    d_whT = nc.dram_tensor("whT", [D, D], F32R, kind="ExternalInput").ap()
    d_nbi = nc.dram_tensor("nbi", [D, 1], F32, kind="ExternalInput").ap()
    d_nbf = nc.dram_tensor("nbf", [D, 1], F32, kind="ExternalInput").ap()
    d_nbh = nc.dram_tensor("nbh", [D, 1], F32, kind="ExternalInput").ap()
    d_bh = nc.dram_tensor("bh", [D, 1], F32, kind="ExternalInput").ap()
    d_h0g = nc.dram_tensor("h0g", [D, 1], F32, kind="ExternalInput").ap()
    d_tri = nc.dram_tensor("tri", [128, 128], F32, kind="ExternalInput").ap()
    d_ones1 = nc.dram_tensor("ones1", [1, 128], F32, kind="ExternalInput").ap()
    d_onesc = nc.dram_tensor("onesc", [128, 1], F32, kind="ExternalInput").ap()
    d_hT = nc.dram_tensor("hT", [D, T], F32, kind="ExternalOutput").ap()

    with tile.TileContext(nc) as tc, ExitStack() as ctx:
        singles = ctx.enter_context(tc.tile_pool(name="singles", bufs=1))
        xpool = ctx.enter_context(tc.tile_pool(name="xpool", bufs=9))
        work = ctx.enter_context(tc.tile_pool(name="work", bufs=2))
        tiny = ctx.enter_context(tc.tile_pool(name="tiny", bufs=2))
        carryp = ctx.enter_context(tc.tile_pool(name="carryp", bufs=2))
        ps_zi = ctx.enter_context(tc.tile_pool(name="ps_zi", bufs=2, space="PSUM"))
        ps_zf = ctx.enter_context(tc.tile_pool(name="ps_zf", bufs=2, space="PSUM"))
        ps_zh = ctx.enter_context(tc.tile_pool(name="ps_zh", bufs=1, space="PSUM"))
        ps_c = ctx.enter_context(tc.tile_pool(name="ps_c", bufs=2, space="PSUM"))
        ps_cr = ctx.enter_context(tc.tile_pool(name="ps_cr", bufs=1, space="PSUM"))

        # ---- resident constants ----
        w_tiles = {}
        for gname, dw in (("i", d_wiT), ("f", d_wfT), ("h", d_whT)):
            for k in range(KT):
                wt = singles.tile([128, D], F32R, tag=f"w{gname}{k}")
                nc.sync.dma_start(out=wt, in_=dw[128 * k:128 * (k + 1), :])
                w_tiles[(gname, k)] = wt
        t_tri = singles.tile([128, 128], F32, tag="tri")
        nc.sync.dma_start(out=t_tri, in_=d_tri)
        t_ones1 = singles.tile([1, 128], F32, tag="ones1")
        nc.sync.dma_start(out=t_ones1, in_=d_ones1)
        t_onesc = singles.tile([128, 1], F32, tag="onesc")
        nc.sync.dma_start(out=t_onesc, in_=d_onesc)
        # biases: load as [128, 8] where col i = o-tile i  (DRAM [D,1] strided)
        t_nbi = singles.tile([128, 8], F32, tag="nbi")
        nc.sync.dma_start(out=t_nbi, in_=d_nbi.rearrange("(i p) one -> p (i one)", p=128))
        t_nbf = singles.tile([128, 8], F32, tag="nbf")
        nc.sync.dma_start(out=t_nbf, in_=d_nbf.rearrange("(i p) one -> p (i one)", p=128))
        t_nbh = singles.tile([128, 8], F32, tag="nbh")
        nc.sync.dma_start(out=t_nbh, in_=d_nbh.rearrange("(i p) one -> p (i one)", p=128))
        t_bh = singles.tile([128, 8], F32, tag="bh")
        nc.sync.dma_start(out=t_bh, in_=d_bh.rearrange("(i p) one -> p (i one)", p=128))
        t_h0g = singles.tile([128, 8], F32, tag="h0g")
        nc.sync.dma_start(out=t_h0g, in_=d_h0g.rearrange("(i p) one -> p (i one)", p=128))
        t_zeros = singles.tile([128, CH], F32, tag="zeros")
        nc.vector.memset(t_zeros, 0.0)
        t_two = singles.tile([128, 1], F32, tag="two")
        nc.vector.memset(t_two, 2.0)
        # per-o-tile persistent state
        t_cmin = singles.tile([128, 8], F32, tag="cmin")    # min_t c (chunk 0)
        t_ncmin = singles.tile([128, 8], F32, tag="ncmin")  # -cmin
        t_slast = singles.tile([128, 8], F32, tag="slast")  # scan carry per o-tile

        reps = int(os.environ.get("KERNEL_REPS", "1"))
        for _rep in range(reps):
          for j in range(NCH):
            t0 = j * CH
            x_k = []
            for k in range(KT):
                xt = xpool.tile([128, CH], F32R, tag="x")
                nc.sync.dma_start(out=xt, in_=d_xT[128 * k:128 * (k + 1), t0:t0 + CH])
                x_k.append(xt)

            carry_ps = ps_cr.tile([1, CH], F32, tag="cr")
            for i in range(OT):
                osl = slice(128 * i, 128 * (i + 1))
                # ---- gates (PE f32r) ----
                zi = ps_zi.tile([128, CH], F32, tag="zi")
                zf = ps_zf.tile([128, CH], F32, tag="zf")
                zh = ps_zh.tile([128, CH], F32, tag="zh")
                for g, zp in (("i", zi), ("f", zf), ("h", zh)):
                    for k in range(KT):
                        nc.tensor.matmul(zp, w_tiles[(g, k)][:, osl], x_k[k],
                                         start=(k == 0), stop=(k == KT - 1))

                # ---- ACT: p,q,u = exp(-(z+b)) ----
                p = work.tile([128, CH], F32, tag="p")
                nc.scalar.activation(p, zi, AF.Exp, bias=t_nbi[:, i:i + 1], scale=-1.0)
                q = work.tile([128, CH], F32, tag="q")
                nc.scalar.activation(q, zf, AF.Exp, bias=t_nbf[:, i:i + 1], scale=-1.0)
                u = work.tile([128, CH], F32, tag="u")
                nc.scalar.activation(u, zh, AF.Exp, bias=t_nbh[:, i:i + 1], scale=-1.0)
                # rh = relu(zh + bh)  (ACT)
                rh = work.tile([128, CH], F32, tag="rh")
                nc.scalar.activation(rh, zh, AF.Relu, bias=t_bh[:, i:i + 1], scale=1.0)

                # ---- gpsimd: s = p + q ; s2 = s + 2 ; um1 = max(u,1)+1 ----
                s = work.tile([128, CH], F32, tag="s")
                nc.gpsimd.tensor_add(s, p, q)
                s2 = work.tile([128, CH], F32, tag="s2")
                nc.gpsimd.tensor_scalar_add(s2, s, 2.0)
                um1 = work.tile([128, CH], F32, tag="um1")
                nc.gpsimd.tensor_scalar(out=um1, in0=u, scalar1=1.0, scalar2=1.0,
                                        op0=ALU.max, op1=ALU.add)

                # ---- ACT: A = ln(1+p); C = ln(s2) ----
                A = work.tile([128, CH], F32, tag="A")
                nc.scalar.activation(A, p, AF.Ln, bias=1.0, scale=1.0)
                Ct = work.tile([128, CH], F32, tag="Ct")
                nc.scalar.activation(Ct, s, AF.Ln, bias=t_two, scale=1.0)

                # ---- DVE: lf = A - C ; recips; i-gate ----
                lf = work.tile([128, CH], F32, tag="lf")
                nc.vector.tensor_sub(lf, A, Ct)
                r1 = work.tile([128, CH], F32, tag="r1")
                nc.vector.reciprocal_approx_fast(out=r1, in_=s2)
                r2 = work.tile([128, CH], F32, tag="r2")
                nc.vector.reciprocal_approx_fast(out=r2, in_=um1)
                ig = work.tile([128, CH], F32, tag="ig")
                nc.vector.scalar_tensor_tensor(out=ig, in0=q, scalar=1.0, in1=r1,
                                               op0=ALU.add, op1=ALU.mult)

                # ---- gpsimd: g = rh + r2 ; iv = ig * g ----
                gt = work.tile([128, CH], F32, tag="gt")
                nc.gpsimd.tensor_add(gt, rh, r2)
                iv = work.tile([128, CH], F32, tag="iv")
                nc.gpsimd.tensor_mul(iv, ig, gt)

                # ---- PE: c = tri-cumsum_d(lf) + carry (fp32, exact data path) ----
                c_ps = ps_c.tile([128, CH], F32, tag="c")
                nc.tensor.matmul(c_ps, t_tri, lf, start=True, stop=(i == 0))
                if i > 0:
                    # carry_ps currently holds sum of colsum(lf_0..lf_{i-1})
                    carry_row = carryp.tile([1, CH], F32, tag="carry")
                    nc.vector.tensor_copy(carry_row, carry_ps)
                    nc.tensor.matmul(c_ps, t_ones1, carry_row, start=False,
                                     stop=True)
                if i < OT - 1:
                    nc.tensor.matmul(carry_ps, t_onesc, lf, start=(i == 0),
                                     stop=(i == OT - 2), skip_group_check=True)

                if j == 0:
                    # m-shift from chunk 0: cmin = min_t c ; ncmin = -cmin
                    nc.vector.tensor_reduce(out=t_cmin[:, i:i + 1], in_=c_ps,
                                            op=ALU.min, axis=AX.X)
                    nc.vector.tensor_scalar_mul(t_ncmin[:, i:i + 1],
                                                t_cmin[:, i:i + 1], -1.0)
                    # scan init: S0 = h0g * exp(cmin)
                    w0 = tiny.tile([128, 1], F32, tag="w0")
                    nc.scalar.activation(w0, t_cmin[:, i:i + 1], AF.Exp)
                    nc.vector.tensor_mul(t_slast[:, i:i + 1], t_h0g[:, i:i + 1], w0)

                # ---- Ey = exp(cmin - c) ; E = Ey * iv ----
                Ey = work.tile([128, CH], F32, tag="Ey")
                nc.scalar.activation(Ey, c_ps, AF.Exp, bias=t_cmin[:, i:i + 1],
                                     scale=-1.0)
                E = work.tile([128, CH], F32, tag="E")
                nc.vector.tensor_mul(E, Ey, iv)

                # ---- scan: S[t] = S0 + sum_{s<=t} E[s] ----
                S = work.tile([128, CH], F32, tag="S")
                nc.vector.tensor_tensor_scan(S, E, t_zeros, t_slast[:, i:i + 1],
                                             ALU.add, ALU.add)
                nc.vector.tensor_copy(t_slast[:, i:i + 1], S[:, CH - 1:CH])

                # ---- h = exp(c - cmin) * S ----
                G = work.tile([128, CH], F32, tag="G")
                nc.scalar.activation(G, c_ps, AF.Exp, bias=t_ncmin[:, i:i + 1],
                                     scale=1.0)
                h = work.tile([128, CH], F32, tag="h")
                nc.vector.tensor_mul(h, G, S)
                nc.sync.dma_start(out=d_hT[osl, t0:t0 + CH], in_=h)

    nc.compile()
    return nc


def _log_g64(x):
    out = np.where(x >= 0, np.log(np.maximum(x, 0.0) + 0.5),
                   -np.log1p(np.exp(-np.abs(x))))
    return out


def kernel(x, h_0, Wi, bi, Wf, bf, Wh, bh):
    global _compiled_nc
    if _compiled_nc is None:
        _compiled_nc = build_nc()
    nc = _compiled_nc

    x = np.asarray(x, np.float32)
    h_0 = np.asarray(h_0, np.float32)
    Wi = np.asarray(Wi, np.float32); bi = np.asarray(bi, np.float32)
    Wf = np.asarray(Wf, np.float32); bf = np.asarray(bf, np.float32)
    Wh = np.asarray(Wh, np.float32); bh = np.asarray(bh, np.float32)

    wiT = np.ascontiguousarray(Wi.T)
    wfT = np.ascontiguousarray(Wf.T)
    whT = np.ascontiguousarray(Wh.T)
    tri = np.triu(np.ones((128, 128), np.float32))
    ones1 = np.ones((1, 128), np.float32)
    onesc = np.ones((128, 1), np.float32)
    nbi = (-bi).reshape(D, 1).astype(np.float32)
    nbf = (-bf).reshape(D, 1).astype(np.float32)
    nbh = (-bh).reshape(D, 1).astype(np.float32)
    bh_c = bh.reshape(D, 1).astype(np.float32)

    h0g_all = np.exp(_log_g64(h_0[:, 0, :].astype(np.float64))).astype(np.float32)

    n_cores = int(os.environ.get("KERNEL_CORES", "8"))
    in_maps = []
    for b in range(n_cores):
        in_maps.append({
            "xT": np.ascontiguousarray(x[b].T),
            "wiT": wiT, "wfT": wfT, "whT": whT,
            "nbi": nbi, "nbf": nbf, "nbh": nbh, "bh": bh_c,
            "h0g": h0g_all[b].reshape(D, 1),
            "tri": tri, "ones1": ones1, "onesc": onesc,
        })

    trace = os.environ.get("KERNEL_TRACE", "0") == "1"
    res = run_bass_kernel_spmd(nc, in_maps, core_ids=list(range(n_cores)),
                               trace=trace)
    if trace and res.exec_time_ns is not None:
        kernel.last_exec_time_ns = res.exec_time_ns
        kernel.last_trace = res.instructions_and_trace
        kernel.last_mean_exec_ns = res.mean_exec_time_ns

    out = np.empty((B, T + 1, D), np.float32)
    for b in range(n_cores):
        hT = res.results[b]["hT"]
        out[b, 0, :] = h0g_all[b]
        out[b, 1:, :] = hT.T
    for b in range(n_cores, B):
        out[b] = 0.0
    return out


# revision 23
# speedup vs baseline: 197.2090x; 197.2090x over previous
"""Trainium2 Bass kernel for nn_MiniLSTMParallelCell.

Data-parallel over batch B=8 across 8 NeuronCores (1 sample/core).

Math (per sample, all on-device in [d_out=partition, t=free] layout):
  zi/zf/zh = x @ W^T + b   (PE, float32r, K=1024 contraction)
  p=e^{-zi}, q=e^{-zf}, u=e^{-zh}           (ACT Exp, bias=-b folded in)
  A = ln(1+p) = softplus(-zi)
  C = ln(2+p+q)
  log_f = A - C; c[d,t] = cumsum_d(log_f)    (PE fp32 triangular + rank-1 carry)
  i_gate = (1+q)/(2+p+q)                     (DVE reciprocal_approx_fast)
  g = relu(zh+bh) + 1/(1+max(u,1))           (= minLSTM log_g in linear space)
  iv = i_gate * g
  h[t] = exp(c[t]+m)*(S0 + sum_{s<=t} exp(-c[s]-m)*iv[s])   m = per-d shift
  time-cumsum via DVE tensor_tensor_scan along the free dim.

Host does: x/W transposes, g(h_0), output detranspose (not part of HW time).
"""
import os
import sys
import numpy as np

sys.path.insert(0, "/opt/trn_rl_repo")

from contextlib import ExitStack

import concourse.bass as bass
import concourse.bacc as bacc
import concourse.tile as tile
import concourse.mybir as mybir
from concourse.bass_utils import run_bass_kernel_spmd

# Force the ACT table chooser onto the combined ln+exp set: the greedy
# set-selection pass otherwise alternates exp_and_others <-> natural_log,
# inserting ~227 table loads (~290us of ACT time). Stripping Exp/Ln from
# every other set (list order/indices preserved) leaves
# natural_log_exp_and_others as the only candidate for both.
_orig_get_tables = bacc.get_activation_tables


def _patched_get_tables(arch):
    tables = _orig_get_tables(arch)
    AFT = mybir.ActivationFunctionType
    out = {}
    for name, funcs in tables.items():
        if name != "natural_log_exp_and_others":
            funcs = funcs - {AFT.Exp, AFT.Ln}
        out[name] = funcs
    return out


bacc.get_activation_tables = _patched_get_tables

F32 = mybir.dt.float32
F32R = mybir.dt.float32r
AF = mybir.ActivationFunctionType
ALU = mybir.AluOpType
AX = mybir.AxisListType

B, T, D = 8, 4096, 1024
CH = 512                  # time chunk (PSUM bank width)
NCH = T // CH             # 8 chunks
OT = D // 128             # 8 output-feature tiles
KT = D // 128             # 8 contraction tiles

_compiled_nc = None


def build_nc():
    nc = bacc.Bacc("TRN2", debug=False, target_bir_lowering=False)

    d_xT = nc.dram_tensor("xT", [D, T], F32R, kind="ExternalInput").ap()
    d_wiT = nc.dram_tensor("wiT", [D, D], F32R, kind="ExternalInput").ap()
    d_wfT = nc.dram_tensor("wfT", [D, D], F32R, kind="ExternalInput").ap()
    d_whT = nc.dram_tensor("whT", [D, D], F32R, kind="ExternalInput").ap()
    d_nbi = nc.dram_tensor("nbi", [D, 1], F32, kind="ExternalInput").ap()
    d_nbf = nc.dram_tensor("nbf", [D, 1], F32, kind="ExternalInput").ap()
    d_nbh = nc.dram_tensor("nbh", [D, 1], F32, kind="ExternalInput").ap()
    d_bh = nc.dram_tensor("bh", [D, 1], F32, kind="ExternalInput").ap()
    d_h0g = nc.dram_tensor("h0g", [D, 1], F32, kind="ExternalInput").ap()
    d_tri = nc.dram_tensor("tri", [128, 128], F32, kind="ExternalInput").ap()
    d_ones1 = nc.dram_tensor("ones1", [1, 128], F32, kind="ExternalInput").ap()
    d_sel127 = nc.dram_tensor("sel127", [32, 128], F32, kind="ExternalInput").ap()
    d_hT = nc.dram_tensor("hT", [D, T], F32, kind="ExternalOutput").ap()

    with tile.TileContext(nc) as tc, ExitStack() as ctx:
        singles = ctx.enter_context(tc.tile_pool(name="singles", bufs=1))
        xpool = ctx.enter_context(tc.tile_pool(name="xpool", bufs=9))
        work = ctx.enter_context(tc.tile_pool(name="work", bufs=2))
        tiny = ctx.enter_context(tc.tile_pool(name="tiny", bufs=2))
        carryp = ctx.enter_context(tc.tile_pool(name="carryp", bufs=2))
        ps_zi = ctx.enter_context(tc.tile_pool(name="ps_zi", bufs=2, space="PSUM"))
        ps_zf = ctx.enter_context(tc.tile_pool(name="ps_zf", bufs=2, space="PSUM"))
        ps_zh = ctx.enter_context(tc.tile_pool(name="ps_zh", bufs=2, space="PSUM"))
        ps_c = ctx.enter_context(tc.tile_pool(name="ps_c", bufs=2, space="PSUM"))

        # ---- resident constants ----
        w_tiles = {}
        for gname, dw in (("i", d_wiT), ("f", d_wfT), ("h", d_whT)):
            for k in range(KT):
                wt = singles.tile([128, D], F32R, tag=f"w{gname}{k}")
                nc.sync.dma_start(out=wt, in_=dw[128 * k:128 * (k + 1), :])
                w_tiles[(gname, k)] = wt
        t_tri = singles.tile([128, 128], F32, tag="tri")
        nc.sync.dma_start(out=t_tri, in_=d_tri)
        t_ones1 = singles.tile([1, 128], F32, tag="ones1")
        nc.sync.dma_start(out=t_ones1, in_=d_ones1)
        t_sel = singles.tile([32, 128], F32, tag="sel127")
        nc.sync.dma_start(out=t_sel, in_=d_sel127)
        # biases: load as [128, 8] where col i = o-tile i  (DRAM [D,1] strided)
        t_nbi = singles.tile([128, 8], F32, tag="nbi")
        nc.sync.dma_start(out=t_nbi, in_=d_nbi.rearrange("(i p) one -> p (i one)", p=128))
        t_nbf = singles.tile([128, 8], F32, tag="nbf")
        nc.sync.dma_start(out=t_nbf, in_=d_nbf.rearrange("(i p) one -> p (i one)", p=128))
        t_nbh = singles.tile([128, 8], F32, tag="nbh")
        nc.sync.dma_start(out=t_nbh, in_=d_nbh.rearrange("(i p) one -> p (i one)", p=128))
        t_bh = singles.tile([128, 8], F32, tag="bh")
        nc.sync.dma_start(out=t_bh, in_=d_bh.rearrange("(i p) one -> p (i one)", p=128))
        t_h0g = singles.tile([128, 8], F32, tag="h0g")
        nc.sync.dma_start(out=t_h0g, in_=d_h0g.rearrange("(i p) one -> p (i one)", p=128))
        t_zeros = singles.tile([128, CH], F32, tag="zeros")
        nc.vector.memset(t_zeros, 0.0)
        t_two = singles.tile([128, 1], F32, tag="two")
        nc.vector.memset(t_two, 2.0)
        # per-o-tile persistent state
        t_cmin = singles.tile([128, 8], F32, tag="cmin")    # min_t c (chunk 0)
        t_ncmin = singles.tile([128, 8], F32, tag="ncmin")  # -cmin
        t_slast = singles.tile([128, 8], F32, tag="slast")  # scan carry per o-tile

        reps = int(os.environ.get("KERNEL_REPS", "1"))
        for _rep in range(reps):
          for j in range(NCH):
            t0 = j * CH
            x_k = []
            for k in range(KT):
                xt = xpool.tile([128, CH], F32R, tag="x")
                nc.sync.dma_start(out=xt, in_=d_xT[128 * k:128 * (k + 1), t0:t0 + CH])
                x_k.append(xt)

            cc_prev = None
            for i in range(OT):
                osl = slice(128 * i, 128 * (i + 1))
                # ---- gates (PE f32r) ----
                zi = ps_zi.tile([128, CH], F32, tag="zi")
                zf = ps_zf.tile([128, CH], F32, tag="zf")
                zh = ps_zh.tile([128, CH], F32, tag="zh")
                for g, zp in (("i", zi), ("f", zf), ("h", zh)):
                    for k in range(KT):
                        nc.tensor.matmul(zp, w_tiles[(g, k)][:, osl], x_k[k],
                                         start=(k == 0), stop=(k == KT - 1))

                # ---- ACT: p,q,u = exp(-(z+b)) ----
                p = work.tile([128, CH], F32, tag="p")
                nc.scalar.activation(p, zi, AF.Exp, bias=t_nbi[:, i:i + 1], scale=-1.0)
                q = work.tile([128, CH], F32, tag="q")
                nc.scalar.activation(q, zf, AF.Exp, bias=t_nbf[:, i:i + 1], scale=-1.0)
                u = work.tile([128, CH], F32, tag="u")
                nc.scalar.activation(u, zh, AF.Exp, bias=t_nbh[:, i:i + 1], scale=-1.0)
                # rh = relu(zh + bh)  (ACT)
                rh = work.tile([128, CH], F32, tag="rh")
                nc.scalar.activation(rh, zh, AF.Relu, bias=t_bh[:, i:i + 1], scale=1.0)

                # ---- gpsimd: s = p + q ; s2 = s + 2 ; um1 = max(u,1)+1 ----
                s = work.tile([128, CH], F32, tag="s")
                nc.gpsimd.tensor_add(s, p, q)
                s2 = work.tile([128, CH], F32, tag="s2")
                nc.gpsimd.tensor_scalar_add(s2, s, 2.0)
                um1 = work.tile([128, CH], F32, tag="um1")
                nc.gpsimd.tensor_scalar(out=um1, in0=u, scalar1=1.0, scalar2=1.0,
                                        op0=ALU.max, op1=ALU.add)

                # ---- ACT: A = ln(1+p); C = ln(s2) ----
                A = work.tile([128, CH], F32, tag="A")
                nc.scalar.activation(A, p, AF.Ln, bias=1.0, scale=1.0)
                Ct = work.tile([128, CH], F32, tag="Ct")
                nc.scalar.activation(Ct, s, AF.Ln, bias=t_two, scale=1.0)

                # ---- DVE: lf = A - C ; recips; i-gate ----
                lf = work.tile([128, CH], F32, tag="lf")
                nc.vector.tensor_sub(lf, A, Ct)
                r1 = work.tile([128, CH], F32, tag="r1")
                nc.vector.reciprocal_approx_fast(out=r1, in_=s2)
                r2 = work.tile([128, CH], F32, tag="r2")
                nc.vector.reciprocal_approx_fast(out=r2, in_=um1)
                ig = work.tile([128, CH], F32, tag="ig")
                nc.vector.scalar_tensor_tensor(out=ig, in0=q, scalar=1.0, in1=r1,
                                               op0=ALU.add, op1=ALU.mult)

                # ---- gpsimd: g = rh + r2 ; iv = ig * g ----
                gt = work.tile([128, CH], F32, tag="gt")
                nc.gpsimd.tensor_add(gt, rh, r2)
                iv = work.tile([128, CH], F32, tag="iv")
                nc.gpsimd.tensor_mul(iv, ig, gt)

                # ---- PE: c = tri-cumsum_d(lf) + carry (fp32, exact data path) ----
                c_ps = ps_c.tile([128, CH], F32, tag="c")
                nc.tensor.matmul(c_ps, t_tri, lf, start=True, stop=(i == 0))
                if i > 0:
                    # broadcast previous tile's last row (full cumsum so far)
                    nc.tensor.matmul(c_ps, t_sel, cc_prev, start=False,
                                     stop=True)
                if i < OT - 1:
                    cc_prev = carryp.tile([32, CH], F32, tag="cc")
                    nc.vector.tensor_copy(cc_prev, c_ps[96:128, :])

                if j == 0:
                    # m-shift from chunk 0: cmin = min_t c ; ncmin = -cmin
                    nc.vector.tensor_reduce(out=t_cmin[:, i:i + 1], in_=c_ps,
                                            op=ALU.min, axis=AX.X)
                    nc.vector.tensor_scalar_mul(t_ncmin[:, i:i + 1],
                                                t_cmin[:, i:i + 1], -1.0)
                    # scan init: S0 = h0g * exp(cmin)
                    w0 = tiny.tile([128, 1], F32, tag="w0")
                    nc.scalar.activation(w0, t_cmin[:, i:i + 1], AF.Exp)
                    nc.vector.tensor_mul(t_slast[:, i:i + 1], t_h0g[:, i:i + 1], w0)

                # ---- Ey = exp(cmin - c) ; E = Ey * iv ----
                Ey = work.tile([128, CH], F32, tag="Ey")
                nc.scalar.activation(Ey, c_ps, AF.Exp, bias=t_cmin[:, i:i + 1],
                                     scale=-1.0)
                E = work.tile([128, CH], F32, tag="E")
                nc.vector.tensor_mul(E, Ey, iv)

                # ---- scan: S[t] = S0 + sum_{s<=t} E[s] ----
                S = work.tile([128, CH], F32, tag="S")
                nc.vector.tensor_tensor_scan(S, E, t_zeros, t_slast[:, i:i + 1],
                                             ALU.add, ALU.add)
                nc.vector.tensor_copy(t_slast[:, i:i + 1], S[:, CH - 1:CH])

                # ---- h = exp(c - cmin) * S ----
                G = work.tile([128, CH], F32, tag="G")
                nc.scalar.activation(G, c_ps, AF.Exp, bias=t_ncmin[:, i:i + 1],
                                     scale=1.0)
                h = work.tile([128, CH], F32, tag="h")
                nc.vector.tensor_mul(h, G, S)
                nc.sync.dma_start(out=d_hT[osl, t0:t0 + CH], in_=h)

    nc.compile()
    return nc


def _log_g64(x):
    out = np.where(x >= 0, np.log(np.maximum(x, 0.0) + 0.5),
                   -np.log1p(np.exp(-np.abs(x))))
    return out


def kernel(x, h_0, Wi, bi, Wf, bf, Wh, bh):
    global _compiled_nc
    if _compiled_nc is None:
        _compiled_nc = build_nc()
    nc = _compiled_nc

    x = np.asarray(x, np.float32)
    h_0 = np.asarray(h_0, np.float32)
    Wi = np.asarray(Wi, np.float32); bi = np.asarray(bi, np.float32)
    Wf = np.asarray(Wf, np.float32); bf = np.asarray(bf, np.float32)
    Wh = np.asarray(Wh, np.float32); bh = np.asarray(bh, np.float32)

    wiT = np.ascontiguousarray(Wi.T)
    wfT = np.ascontiguousarray(Wf.T)
    whT = np.ascontiguousarray(Wh.T)
    tri = np.triu(np.ones((128, 128), np.float32))
    ones1 = np.ones((1, 128), np.float32)
    sel127 = np.zeros((32, 128), np.float32)
    sel127[31, :] = 1.0
    nbi = (-bi).reshape(D, 1).astype(np.float32)
    nbf = (-bf).reshape(D, 1).astype(np.float32)
    nbh = (-bh).reshape(D, 1).astype(np.float32)
    bh_c = bh.reshape(D, 1).astype(np.float32)

    h0g_all = np.exp(_log_g64(h_0[:, 0, :].astype(np.float64))).astype(np.float32)

    n_cores = int(os.environ.get("KERNEL_CORES", "8"))
    in_maps = []
    for b in range(n_cores):
        in_maps.append({
            "xT": np.ascontiguousarray(x[b].T),
            "wiT": wiT, "wfT": wfT, "whT": whT,
            "nbi": nbi, "nbf": nbf, "nbh": nbh, "bh": bh_c,
            "h0g": h0g_all[b].reshape(D, 1),
            "tri": tri, "ones1": ones1, "sel127": sel127,
        })

    trace = os.environ.get("KERNEL_TRACE", "0") == "1"
    res = run_bass_kernel_spmd(nc, in_maps, core_ids=list(range(n_cores)),
                               trace=trace)
    if trace and res.exec_time_ns is not None:
        kernel.last_exec_time_ns = res.exec_time_ns
        kernel.last_trace = res.instructions_and_trace
        kernel.last_mean_exec_ns = res.mean_exec_time_ns

    out = np.empty((B, T + 1, D), np.float32)
    for b in range(n_cores):
        hT = res.results[b]["hT"]
        out[b, 0, :] = h0g_all[b]
        out[b, 1:, :] = hT.T
    for b in range(n_cores, B):
        out[b] = 0.0
    return out
